# revision 1
# baseline (speedup 1.0000x reference)
"""Trainium2 Bass kernel for CaMoE (LN + top-2 MoE with relu^2 FFN).

Strategy: expert-parallel over 8 NeuronCores. Core e receives only the
tokens routed to expert e (gather indices computed host-side as part of
sharding), plus W1[e]/W2[e] in bf16, pre-swizzled into per-tile lhsT
layout. On device: LayerNorm stats via ones-matmul in replicated-lane
form (stats vectors come out already broadcast across partitions),
xn = (x - mu) * rstd * sqrt(coef) in bf16, hidden = relu(xn @ W1)^2
with fp32 PSUM accumulation, y = hidden @ W2, written back
feature-major. Host scatter-adds the 8 partial outputs into x (the
residual) — pure unsharding, no collectives needed.

Self-contained: hardcodes shapes B=4, T=2048, C=1024, E=8, H=4096.
"""

import os
import sys

for _p in ("/opt/trn_rl_repo", "/root/.axon_site/_ro/trn_rl_repo"):
    if os.path.isdir(_p) and _p not in sys.path:
        sys.path.insert(0, _p)

from contextlib import ExitStack

import ml_dtypes
import numpy as np

import concourse.bass as bass
import concourse.tile as tile
from concourse import bacc, mybir
from concourse.bass_utils import run_bass_kernel_spmd

N_CORES = 8
C = 1024
H = 4096
NB = 512          # token block (matmul moving free dim)
NC_T = C // 128   # 8 c-tiles
NH_T = H // 128   # 32 h-tiles
EPS = 1e-5

F32 = mybir.dt.float32
BF16 = mybir.dt.bfloat16
AF = mybir.ActivationFunctionType
OP = mybir.AluOpType


def _build_kernel(NT: int, has_beta: bool):
    """Build the per-core SPMD program for NT padded tokens."""
    blocks = []
    t0 = 0
    while t0 < NT:
        tn = min(NB, NT - t0)
        blocks.append((t0, tn))
        t0 += tn
    nblk = len(blocks)
    nc = bacc.Bacc("TRN2", target_bir_lowering=False, debug=False, num_devices=1)

    xgt_d = nc.dram_tensor("xgt", [C, NT], F32, kind="ExternalInput").ap()
    # weights pre-swizzled on host into per-tile lhsT layout:
    #   w1[h][p, c*128+j] = (gamma*W1)[c*128+p, h*128+j]
    #   w2[c][p, h*128+j] = W2[h*128+p, c*128+j]
    w1_d = nc.dram_tensor("w1", [NH_T, 128, C], BF16, kind="ExternalInput").ap()
    w2_d = nc.dram_tensor("w2", [NC_T, 128, H], BF16, kind="ExternalInput").ap()
    cg_d = nc.dram_tensor("cg", [1, NT], F32, kind="ExternalInput").ap()
    if has_beta:
        bias1_d = nc.dram_tensor("bias1", [128, NH_T], F32, kind="ExternalInput").ap()
    ygt_d = nc.dram_tensor("ygt", [C, NT], F32, kind="ExternalOutput").ap()

    with tile.TileContext(nc) as tc, ExitStack() as ctx:
        sb = ctx.enter_context(tc.tile_pool(name="sb", bufs=1))
        ps = ctx.enter_context(tc.tile_pool(name="ps", bufs=1, space="PSUM"))

        # ---- constants ----
        ones_k = sb.tile([128, 128], BF16, tag="ones_k", bufs=1)
        nc.vector.memset(ones_k, 1.0)
        eps_t = sb.tile([128, 1], F32, tag="eps", bufs=1)
        nc.vector.memset(eps_t, EPS)
        if has_beta:
            b1sb = sb.tile([128, NH_T], F32, tag="b1", bufs=1)
            nc.sync.dma_start(b1sb, bias1_d)

        def stats_phase(blk):
            """LN stats for block blk, replicated-lane form.

            Returns [128,tn] scale/shift (already broadcast across
            partitions) plus the raw x tiles (kept for normalize)."""
            t0, tn = blocks[blk]
            tsl = bass.ds(t0, tn)
            sum_ps = ps.tile([128, tn], F32, tag="stat", bufs=3, name=f"sum{blk}")
            sq_ps = ps.tile([128, tn], F32, tag="stat", bufs=3, name=f"sq{blk}")
            xs = []
            for c in range(NC_T):
                xt = sb.tile([128, tn], F32, tag="xs", bufs=14, name=f"xa{blk}_{c}", padded_shape=[128, NB])
                nc.sync.dma_start(xt, xgt_d[c * 128:(c + 1) * 128, tsl])
                xb = sb.tile([128, tn], BF16, tag="xb16", bufs=3, name=f"xb16{blk}_{c}", padded_shape=[128, NB])
                nc.vector.tensor_copy(xb, xt)
                xsq = sb.tile([128, tn], BF16, tag="xsq", bufs=3, name=f"xsq{blk}_{c}", padded_shape=[128, NB])
                nc.scalar.activation(xsq, xt, AF.Square)
                nc.tensor.matmul(sum_ps, ones_k, xb,
                                 start=(c == 0), stop=(c == NC_T - 1))
                nc.tensor.matmul(sq_ps, ones_k, xsq,
                                 start=(c == 0), stop=(c == NC_T - 1))
                xs.append(xt)
            vmu = sb.tile([128, tn], F32, tag="vec", bufs=3, name=f"vmu{blk}", padded_shape=[128, NB])
            nc.vector.tensor_scalar_mul(vmu, sum_ps, 1.0 / C)
            # var = sq/C - mu^2
            vvar = sb.tile([128, tn], F32, tag="vec", bufs=3, name=f"vvar{blk}", padded_shape=[128, NB])
            nc.vector.scalar_tensor_tensor(vvar, vmu, -1.0, vmu, OP.mult, OP.mult)
            nc.vector.scalar_tensor_tensor(vvar, sq_ps, 1.0 / C, vvar, OP.mult, OP.add)
            vstd = sb.tile([128, tn], F32, tag="vec", bufs=3, name=f"vstd{blk}", padded_shape=[128, NB])
            nc.scalar.activation(vstd, vvar, AF.Sqrt, bias=eps_t)
            vrstd = sb.tile([128, tn], F32, tag="vec", bufs=3, name=f"vrstd{blk}", padded_shape=[128, NB])
            nc.vector.reciprocal_approx_fast(out=vrstd, in_=vstd)
            vcg = sb.tile([128, tn], F32, tag="bc", bufs=8, name=f"vcg{blk}", padded_shape=[128, NB])
            nc.sync.dma_start(vcg, cg_d[0:1, tsl].to_broadcast([128, tn]))
            if has_beta:
                vs = vrstd          # coef applied on the output instead
            else:
                vs = sb.tile([128, tn], F32, tag="bc", bufs=8, name=f"vs{blk}", padded_shape=[128, NB])
                nc.vector.tensor_mul(vs, vrstd, vcg)
            vb = sb.tile([128, tn], F32, tag="bc", bufs=8, name=f"vb{blk}", padded_shape=[128, NB])
            nc.vector.scalar_tensor_tensor(vb, vmu, -1.0, vs, OP.mult, OP.mult)
            return vs, vb, vcg, xs

        def normalize_phase(blk, vs, vb, xs):
            t0, tn = blocks[blk]
            xn = []
            for c in range(NC_T):
                xt = xs[c]
                nc.vector.tensor_mul(xt, xt, vs)
                xnc = sb.tile([128, tn], BF16, tag="xn", bufs=20, name=f"xn{blk}_{c}", padded_shape=[128, NB])
                nc.vector.tensor_add(xnc, xt, vb)
                xn.append(xnc)
            return xn

        def mm1_phase(blk, xn, mid_hook=None):
            t0, tn = blocks[blk]
            hid = []
            for h in range(NH_T):
                if h == 16 and mid_hook is not None:
                    mid_hook()
                w1t = sb.tile([128, C], BF16, tag="w1s", bufs=8, name=f"w1t{blk}_{h}")
                nc.scalar.dma_start(w1t, w1_d[h])
                pa = ps.tile([128, tn], F32, tag="mm", bufs=4, name=f"pa{blk}_{h}")
                for c in range(NC_T):
                    nc.tensor.matmul(pa, w1t[:, c * 128:(c + 1) * 128], xn[c],
                                     start=(c == 0), stop=(c == NC_T - 1))
                if has_beta:
                    nc.vector.tensor_scalar_add(pa, pa, b1sb[:, h:h + 1])
                # relu(x)^2 == max(x,0)*x; DVE may read only one PSUM operand
                rt = sb.tile([128, tn], BF16, tag="rt", bufs=3, name=f"r{blk}_{h}", padded_shape=[128, NB])
                nc.vector.tensor_scalar_max(rt, pa, 0.0)
                ht = sb.tile([128, tn], BF16, tag="hid", bufs=44, name=f"h{blk}_{h}", padded_shape=[128, NB])
                nc.vector.tensor_mul(ht, rt, pa)
                hid.append(ht)
            return hid

        def mm2_phase(blk, hid, vcf):
            t0, tn = blocks[blk]
            tsl = bass.ds(t0, tn)
            for c in range(NC_T):
                w2t = sb.tile([128, H], BF16, tag="w2s", bufs=4, name=f"w2t{blk}_{c}")
                nc.scalar.dma_start(w2t, w2_d[c])
                pb = ps.tile([128, tn], F32, tag="mm", bufs=4, name=f"pb{blk}_{c}")
                for h in range(NH_T):
                    nc.tensor.matmul(pb, w2t[:, h * 128:(h + 1) * 128], hid[h],
                                     start=(h == 0), stop=(h == NH_T - 1))
                ot = sb.tile([128, tn], F32, tag="out", bufs=4, name=f"o{blk}_{c}", padded_shape=[128, NB])
                if has_beta:
                    nc.vector.tensor_mul(ot, pb, vcf)
                else:
                    nc.vector.tensor_copy(ot, pb)
                nc.sync.dma_start(ygt_d[c * 128:(c + 1) * 128, tsl], ot)

        # Software pipeline: stats/normalize of blk+1 are emitted so the PE
        # runs them inside blk's mm1/mm2 stream with no gaps.
        vs0, vb0, vcf, xs0 = stats_phase(0)
        xn = normalize_phase(0, vs0, vb0, xs0)
        nxt = {}
        for blk in range(nblk):
            def mid_hook(b=blk):
                nxt.update(zip(("vs", "vb", "vcf", "xs"), stats_phase(b + 1)))
            hid = mm1_phase(blk, xn, mid_hook if blk + 1 < nblk else None)
            if blk + 1 < nblk:
                xn = normalize_phase(blk + 1, nxt["vs"], nxt["vb"], nxt["xs"])
            mm2_phase(blk, hid, vcf)
            if blk + 1 < nblk:
                vcf = nxt["vcf"]

    nc.compile()
    return nc


_KERNEL_CACHE = {}


def _get_kernel(NT: int, has_beta: bool):
    key = (NT, has_beta)
    if key not in _KERNEL_CACHE:
        _KERNEL_CACHE[key] = _build_kernel(NT, has_beta)
    return _KERNEL_CACHE[key]


def kernel(x, weights, gamma, beta, W1, W2, winners):
    x = np.asarray(x, dtype=np.float32)
    weights = np.asarray(weights, dtype=np.float32)
    gamma = np.asarray(gamma, dtype=np.float32)
    beta = np.asarray(beta, dtype=np.float32)
    W1 = np.asarray(W1, dtype=np.float32)
    W2 = np.asarray(W2, dtype=np.float32)
    winners = np.asarray(winners)

    B, T, C_ = x.shape
    E = W1.shape[0]
    assert C_ == C and E == N_CORES and W1.shape[2] == H

    x_flat = x.reshape(-1, C)
    win = winners.reshape(-1, 2)
    wts = weights.reshape(-1, 2)

    has_beta = bool(np.any(beta != 0.0))

    # ---- host-side routing (sharding prep) ----
    idxs, coefs = [], []
    for e in range(E):
        m = win == e
        tok = np.nonzero(m.any(axis=1))[0]
        cf = (wts * m).sum(axis=1)[tok]
        idxs.append(tok)
        coefs.append(cf.astype(np.float32))
    NT = int(np.ceil(max(len(t) for t in idxs) / 8) * 8)

    in_maps = []
    for e in range(E):
        tok, cf = idxs[e], coefs[e]
        n = len(tok)
        xg = np.zeros((NT, C), np.float32)
        xg[:n] = x_flat[tok]
        cg = np.zeros((1, NT), np.float32)
        # no beta: fold sqrt(coef) into the LN scale (relu^2 is 2-homogeneous
        # and W2 linear, so scaling xn by sqrt(c) scales the output by c).
        cg[0, :n] = cf if has_beta else np.sqrt(cf)
        w1g = (W1[e] * gamma[:, None]).astype(ml_dtypes.bfloat16)
        w1r = np.ascontiguousarray(
            w1g.reshape(NC_T, 128, NH_T, 128).transpose(2, 1, 0, 3)
        ).reshape(NH_T, 128, C)
        w2r = np.ascontiguousarray(
            W2[e].astype(ml_dtypes.bfloat16)
            .reshape(NH_T, 128, NC_T, 128).transpose(2, 1, 0, 3)
        ).reshape(NC_T, 128, H)
        m = {
            "xgt": np.ascontiguousarray(xg.T),
            "w1": w1r,
            "w2": w2r,
            "cg": cg,
        }
        if has_beta:
            b1 = (beta @ W1[e]).astype(np.float32)          # [H]
            m["bias1"] = np.ascontiguousarray(b1.reshape(NH_T, 128).T)
        in_maps.append(m)

    nc = _get_kernel(NT, has_beta)
    res = run_bass_kernel_spmd(nc, in_maps, list(range(N_CORES)))

    # ---- host-side unshard: scatter-add partial expert outputs ----
    out = x_flat.copy()
    for e in range(E):
        yg = res.results[e]["ygt"]                          # [C, NT]
        n = len(idxs[e])
        out[idxs[e]] += yg.T[:n]
    return out.reshape(B, T, C).astype(np.float32)



# revision 14
# speedup vs baseline: 1.0524x; 1.0524x over previous
"""Trainium2 Bass kernel for CaMoE (LN + top-2 MoE with relu^2 FFN).

Strategy: expert-parallel over 8 NeuronCores. Core e receives only the
tokens routed to expert e (gathered host-side), sorted by DESCENDING
combine coefficient, plus W1[e]/W2[e] pre-swizzled into per-tile lhsT
layout. Low-coefficient token blocks run their matmuls in fp8-e4m3 with
perf_mode=DoubleRow (2 K-subtiles per instruction, ~1.8x the bf16 PE
rate); high-coefficient blocks stay bf16. The routing coefficient folds
into the LN scale as sqrt(coef) (relu^2 is 2-homogeneous), so a block's
fp8 quantization error is damped by its (small) coef - that keeps the
absmax error within budget while ~half the FLOPs run at fp8 rate.

LayerNorm stats come from ones-matmuls of fp8 copies of x (DoubleRow as
well); their contribution to the error is negligible. On-chip engines
are balanced: ReLU on ScalarE (with the fp8 scale folded), the square +
down-cast and the normalize on VectorE in 2x bf16 mode, matmuls on PE.

Host scatter-adds the 8 partial outputs into x (the residual) - pure
unsharding, no collectives.

Self-contained: hardcodes shapes B=4, T=2048, C=1024, E=8, H=4096.
"""

import os
import sys

for _p in ("/opt/trn_rl_repo", "/root/.axon_site/_ro/trn_rl_repo"):
    if os.path.isdir(_p) and _p not in sys.path:
        sys.path.insert(0, _p)

from contextlib import ExitStack

import ml_dtypes
import numpy as np

import concourse.bass as bass
import concourse.tile as tile
from concourse import bacc, mybir
from concourse.bass_utils import run_bass_kernel_spmd

N_CORES = 8
C = 1024
H = 4096
NB = 512          # token block (matmul moving free dim)
NC_T = C // 128   # 8 c-tiles
NH_T = H // 128   # 32 h-tiles
EPS = 1e-5
SW = 64.0         # fp8 weight scale (both W1 and W2)
# fp8 tier: rt = sqrt(2)/SW * relu(pa) so ht = rt^2 = 2*h; out = pb/(2*SW)

F32 = mybir.dt.float32
BF16 = mybir.dt.bfloat16
FP8 = mybir.dt.float8e4
AF = mybir.ActivationFunctionType
OP = mybir.AluOpType
DR = mybir.MatmulPerfMode.DoubleRow

# number of trailing (lowest-coef) 512-token blocks run fully in fp8, and
# number of "mm2"-tier blocks (bf16 mm1 + fp8 mm2) just before those
N_FP8_BLOCKS = 2
N_MM2_BLOCKS = 0


def _build_kernel(NT: int, tiers: tuple, has_beta: bool):
    """Build the per-core SPMD program for NT padded tokens.

    tiers[b] in {"bf16", "full"}: precision of block b's matmuls.
    """
    blocks = []
    t0 = 0
    while t0 < NT:
        tn = min(NB, NT - t0)
        blocks.append((t0, tn))
        t0 += tn
    nblk = len(blocks)
    assert len(tiers) == nblk
    any_f8_1 = any(t == "full" for t in tiers)
    any_f8_2 = any(t in ("full", "mm2") for t in tiers)
    any_bf_1 = any(t in ("bf16", "mm2") for t in tiers)
    any_bf_2 = any(t == "bf16" for t in tiers)

    nc = bacc.Bacc("TRN2", target_bir_lowering=False, debug=False, num_devices=1)

    xgt_d = nc.dram_tensor("xgt", [C, NT], F32, kind="ExternalInput").ap()
    # weights pre-swizzled on host into per-tile lhsT layout:
    #   w1[h][p, c, j] = (gamma*W1)[c*128+p, h*128+j]   (fp8 copy scaled by SW)
    #   w2[c][p, h, j] = W2[h*128+p, c*128+j]
    if any_bf_1:
        w1b_d = nc.dram_tensor("w1b", [NH_T, 128, NC_T, 128], BF16,
                               kind="ExternalInput").ap()
    if any_bf_2:
        w2b_d = nc.dram_tensor("w2b", [NC_T, 128, NH_T, 128], BF16,
                               kind="ExternalInput").ap()
    if any_f8_1:
        w1f_d = nc.dram_tensor("w1f", [NH_T, 128, NC_T, 128], FP8,
                               kind="ExternalInput").ap()
    if any_f8_2:
        w2f_d = nc.dram_tensor("w2f", [NC_T, 128, NH_T, 128], FP8,
                               kind="ExternalInput").ap()
    cg_d = nc.dram_tensor("cg", [1, NT], F32, kind="ExternalInput").ap()
    if has_beta:
        bias1_d = nc.dram_tensor("bias1", [128, NH_T], F32, kind="ExternalInput").ap()
    ygt_d = nc.dram_tensor("ygt", [C, NT], F32, kind="ExternalOutput").ap()

    with tile.TileContext(nc) as tc, ExitStack() as ctx:
        sb = ctx.enter_context(tc.tile_pool(name="sb", bufs=1))
        ps = ctx.enter_context(tc.tile_pool(name="ps", bufs=1, space="PSUM"))

        # ---- constants ----
        ones8 = sb.tile([128, 2, 128], FP8, tag="ones8", bufs=1)
        nc.vector.memset(ones8, 1.0)
        eps_t = sb.tile([128, 1], F32, tag="eps", bufs=1)
        nc.vector.memset(eps_t, EPS)
        if has_beta:
            b1sb = sb.tile([128, NH_T], F32, tag="b1", bufs=1)
            nc.sync.dma_start(b1sb, bias1_d)

        def stats_phase(blk):
            """LN stats for block blk via fp8 DoubleRow ones-matmuls.

            Returns bf16 [128,tn] scale/shift (broadcast across
            partitions) plus the raw bf16 x tiles (kept for normalize)."""
            t0, tn = blocks[blk]
            tsl = bass.ds(t0, tn)
            sum_ps = ps.tile([128, tn], F32, tag="stat", bufs=3, name=f"sum{blk}")
            sq_ps = ps.tile([128, tn], F32, tag="stat", bufs=3, name=f"sq{blk}")
            xb = sb.tile([128, NC_T, tn], FP8, tag="xb", bufs=2, name=f"xb{blk}",
                         padded_shape=[128, NC_T, NB])
            xq = sb.tile([128, NC_T, tn], FP8, tag="xq", bufs=2, name=f"xq{blk}",
                         padded_shape=[128, NC_T, NB])
            xs = []
            for c in range(NC_T):
                xt = sb.tile([128, tn], F32, tag="xs", bufs=16, name=f"xa{blk}_{c}",
                             padded_shape=[128, NB])
                nc.sync.dma_start(xt, xgt_d[c * 128:(c + 1) * 128, tsl])
                nc.vector.tensor_copy(xb[:, c, :], xt)
                nc.scalar.activation(xq[:, c, :], xt, AF.Square)
                xs.append(xt)
            for i in range(NC_T // 2):
                pr = bass.ds(2 * i, 2)
                nc.tensor.matmul(sum_ps, ones8, xb[:, pr, :], perf_mode=DR,
                                 start=(i == 0), stop=(i == NC_T // 2 - 1))
            for i in range(NC_T // 2):
                pr = bass.ds(2 * i, 2)
                nc.tensor.matmul(sq_ps, ones8, xq[:, pr, :], perf_mode=DR,
                                 start=(i == 0), stop=(i == NC_T // 2 - 1))
            vmu = sb.tile([128, tn], F32, tag="vec", bufs=3, name=f"vmu{blk}",
                          padded_shape=[128, NB])
            nc.vector.tensor_scalar_mul(vmu, sum_ps, 1.0 / C)
            # var = sq/C - mu^2
            vvar = sb.tile([128, tn], F32, tag="vec", bufs=3, name=f"vvar{blk}",
                           padded_shape=[128, NB])
            nc.vector.scalar_tensor_tensor(vvar, vmu, -1.0, vmu, OP.mult, OP.mult)
            nc.vector.scalar_tensor_tensor(vvar, sq_ps, 1.0 / C, vvar, OP.mult, OP.add)
            vstd = sb.tile([128, tn], F32, tag="vec", bufs=3, name=f"vstd{blk}",
                           padded_shape=[128, NB])
            nc.scalar.activation(vstd, vvar, AF.Sqrt, bias=eps_t)
            vrstd = sb.tile([128, tn], F32, tag="vec", bufs=3, name=f"vrstd{blk}",
                            padded_shape=[128, NB])
            nc.vector.reciprocal_approx_fast(out=vrstd, in_=vstd)
            vcg = sb.tile([128, tn], F32, tag="bc", bufs=4, name=f"vcg{blk}",
                          padded_shape=[128, NB])
            nc.sync.dma_start(vcg, cg_d[0:1, tsl].to_broadcast([128, tn]))
            if has_beta:
                vs = vrstd                         # coef applied on the output
            else:
                vs = sb.tile([128, tn], F32, tag="bc", bufs=4, name=f"vs{blk}",
                             padded_shape=[128, NB])
                nc.vector.tensor_mul(vs, vrstd, vcg)
            vb = sb.tile([128, tn], F32, tag="bc", bufs=4, name=f"vb{blk}",
                         padded_shape=[128, NB])
            nc.vector.scalar_tensor_tensor(vb, vmu, -1.0, vs, OP.mult, OP.mult)
            return vs, vb, vcg, xs

        def normalize_phase(blk, vs, vb, xs):
            t0, tn = blocks[blk]
            f8 = tiers[blk] == "full"            # mm1 precision
            xn = sb.tile([128, NC_T, tn], FP8 if f8 else BF16,
                         tag="xn8" if f8 else "xnb", bufs=2, name=f"xn{blk}",
                         padded_shape=[128, NC_T, NB])
            for c in range(NC_T):
                xt = xs[c]
                nc.vector.tensor_mul(xt, xt, vs)
                nc.vector.tensor_add(xn[:, c, :], xt, vb)
            return xn

        def mm1_phase(blk, xn, mid_hook=None):
            t0, tn = blocks[blk]
            f8_1 = tiers[blk] == "full"
            f8_2 = tiers[blk] in ("full", "mm2")
            hid = sb.tile([128, NH_T, tn], FP8 if f8_2 else BF16,
                          tag="hid8" if f8_2 else "hidb", bufs=1, name=f"hid{blk}",
                          padded_shape=[128, NH_T, NB])
            for h in range(NH_T):
                if h == 16 and mid_hook is not None:
                    mid_hook()
                pa = ps.tile([128, tn], F32, tag="mm", bufs=4, name=f"pa{blk}_{h}")
                if f8_1:
                    w1t = sb.tile([128, NC_T, 128], FP8, tag="w1f", bufs=4,
                                  name=f"w1f{blk}_{h}")
                    nc.scalar.dma_start(w1t, w1f_d[h])
                    for i in range(NC_T // 2):
                        nc.tensor.matmul(pa, w1t[:, bass.ds(2 * i, 2), :],
                                         xn[:, bass.ds(2 * i, 2), :], perf_mode=DR,
                                         start=(i == 0), stop=(i == NC_T // 2 - 1))
                else:
                    w1t = sb.tile([128, NC_T, 128], BF16, tag="w1b", bufs=4,
                                  name=f"w1b{blk}_{h}")
                    nc.scalar.dma_start(w1t, w1b_d[h])
                    for c in range(NC_T):
                        nc.tensor.matmul(pa, w1t[:, c, :], xn[:, c, :],
                                         start=(c == 0), stop=(c == NC_T - 1))
                if has_beta:
                    nc.vector.tensor_scalar_add(pa, pa, b1sb[:, h:h + 1])
                rt = sb.tile([128, tn], BF16, tag="rt", bufs=3, name=f"r{blk}_{h}",
                             padded_shape=[128, NB])
                # fp8 tiers: rt = sqrt(2)*relu(a) so hid = rt^2 = 2h (in fp8
                # range); mm1-fp8 additionally divides out the W1 scale SW.
                rs = 1.0 if not f8_2 else (np.sqrt(2.0) / SW if f8_1 else np.sqrt(2.0))
                nc.scalar.activation(rt, pa, AF.Relu, scale=rs)
                nc.vector.tensor_mul(hid[:, h, :], rt, rt)
            return hid

        def mm2_phase(blk, hid, vcf):
            t0, tn = blocks[blk]
            tsl = bass.ds(t0, tn)
            f8 = tiers[blk] in ("full", "mm2")
            for c in range(NC_T):
                pb = ps.tile([128, tn], F32, tag="mm", bufs=4, name=f"pb{blk}_{c}")
                if f8:
                    w2t = sb.tile([128, NH_T, 128], FP8, tag="w2f", bufs=2,
                                  name=f"w2f{blk}_{c}")
                    nc.scalar.dma_start(w2t, w2f_d[c])
                    for j in range(NH_T // 2):
                        nc.tensor.matmul(pb, w2t[:, bass.ds(2 * j, 2), :],
                                         hid[:, bass.ds(2 * j, 2), :], perf_mode=DR,
                                         start=(j == 0), stop=(j == NH_T // 2 - 1))
                else:
                    w2t = sb.tile([128, NH_T, 128], BF16, tag="w2b", bufs=2,
                                  name=f"w2b{blk}_{c}")
                    nc.scalar.dma_start(w2t, w2b_d[c])
                    for j in range(NH_T):
                        nc.tensor.matmul(pb, w2t[:, j, :], hid[:, j, :],
                                         start=(j == 0), stop=(j == NH_T - 1))
                ot = sb.tile([128, tn], F32, tag="out", bufs=4, name=f"o{blk}_{c}",
                             padded_shape=[128, NB])
                oscale = 1.0 / (2.0 * SW) if f8 else 1.0
                if has_beta:
                    nc.vector.scalar_tensor_tensor(ot, pb, oscale, vcf,
                                                   OP.mult, OP.mult)
                else:
                    nc.vector.tensor_scalar_mul(ot, pb, oscale)
                nc.sync.dma_start(ygt_d[c * 128:(c + 1) * 128, tsl], ot)

        # Software pipeline: stats/normalize of blk+1 are emitted so the PE
        # runs them inside blk's mm1/mm2 stream with no gaps.
        vs0, vb0, vcf, xs0 = stats_phase(0)
        xn = normalize_phase(0, vs0, vb0, xs0)
        nxt = {}
        for blk in range(nblk):
            def mid_hook(b=blk):
                nxt.update(zip(("vs", "vb", "vcf", "xs"), stats_phase(b + 1)))
            hid = mm1_phase(blk, xn, mid_hook if blk + 1 < nblk else None)
            if blk + 1 < nblk:
                xn = normalize_phase(blk + 1, nxt["vs"], nxt["vb"], nxt["xs"])
            mm2_phase(blk, hid, vcf)
            if blk + 1 < nblk:
                vcf = nxt["vcf"]

    nc.compile()
    return nc


_KERNEL_CACHE = {}


def _get_kernel(NT: int, tiers: tuple, has_beta: bool):
    key = (NT, tiers, has_beta)
    if key not in _KERNEL_CACHE:
        _KERNEL_CACHE[key] = _build_kernel(NT, tiers, has_beta)
    return _KERNEL_CACHE[key]


def _swizzle_w1(w, dtype):
    # [C, H] -> [NH_T, 128, NC_T, 128] with [h][p, c, j] = w[c*128+p, h*128+j]
    return np.ascontiguousarray(
        w.reshape(NC_T, 128, NH_T, 128).transpose(2, 1, 0, 3)
    ).astype(dtype)


def _swizzle_w2(w, dtype):
    # [H, C] -> [NC_T, 128, NH_T, 128] with [c][p, h, j] = w[h*128+p, c*128+j]
    return np.ascontiguousarray(
        w.reshape(NH_T, 128, NC_T, 128).transpose(2, 1, 0, 3)
    ).astype(dtype)


def kernel(x, weights, gamma, beta, W1, W2, winners):
    x = np.asarray(x, dtype=np.float32)
    weights = np.asarray(weights, dtype=np.float32)
    gamma = np.asarray(gamma, dtype=np.float32)
    beta = np.asarray(beta, dtype=np.float32)
    W1 = np.asarray(W1, dtype=np.float32)
    W2 = np.asarray(W2, dtype=np.float32)
    winners = np.asarray(winners)

    B, T, C_ = x.shape
    E = W1.shape[0]
    assert C_ == C and E == N_CORES and W1.shape[2] == H

    x_flat = x.reshape(-1, C)
    win = winners.reshape(-1, 2)
    wts = weights.reshape(-1, 2)

    has_beta = bool(np.any(beta != 0.0))

    # ---- host-side routing (sharding prep) ----
    idxs, coefs = [], []
    for e in range(E):
        m = win == e
        tok = np.nonzero(m.any(axis=1))[0]
        cf = (wts * m).sum(axis=1)[tok]
        order = np.argsort(-cf, kind="stable")   # descending coef
        idxs.append(tok[order])
        coefs.append(cf[order].astype(np.float32))
    NT = int(np.ceil(max(len(t) for t in idxs) / 8) * 8)
    nblk = (NT + NB - 1) // NB

    # trailing (low-coef) blocks in fp8, unless beta forces plain path
    n_f8 = 0 if has_beta else min(N_FP8_BLOCKS, nblk)
    n_m2 = 0 if has_beta else min(N_MM2_BLOCKS, nblk - n_f8)
    tiers = tuple(["bf16"] * (nblk - n_f8 - n_m2) + ["mm2"] * n_m2
                  + ["full"] * n_f8)
    any_f8_1 = "full" in tiers
    any_f8_2 = n_f8 + n_m2 > 0
    any_bf_1 = nblk - n_f8 > 0
    any_bf_2 = nblk - n_f8 - n_m2 > 0

    in_maps = []
    for e in range(E):
        tok, cf = idxs[e], coefs[e]
        n = len(tok)
        xg = np.zeros((NT, C), np.float32)
        xg[:n] = x_flat[tok]
        cg = np.zeros((1, NT), np.float32)
        # no beta: fold sqrt(coef) into the LN scale (relu^2 is 2-homogeneous
        # and W2 linear, so scaling xn by sqrt(c) scales the output by c).
        cg[0, :n] = cf if has_beta else np.sqrt(cf)
        w1g = W1[e] * gamma[:, None]
        m = {
            "xgt": np.ascontiguousarray(xg.T),
            "cg": cg,
        }
        if any_bf_1:
            m["w1b"] = _swizzle_w1(w1g, ml_dtypes.bfloat16)
        if any_bf_2:
            m["w2b"] = _swizzle_w2(W2[e], ml_dtypes.bfloat16)
        if any_f8_1:
            m["w1f"] = _swizzle_w1(w1g * SW, ml_dtypes.float8_e4m3)
        if any_f8_2:
            m["w2f"] = _swizzle_w2(W2[e] * SW, ml_dtypes.float8_e4m3)
        if has_beta:
            b1 = (beta @ W1[e]).astype(np.float32)          # [H]
            m["bias1"] = np.ascontiguousarray(b1.reshape(NH_T, 128).T)
        in_maps.append(m)

    nc = _get_kernel(NT, tiers, has_beta)
    res = run_bass_kernel_spmd(nc, in_maps, list(range(N_CORES)))

    # ---- host-side unshard: scatter-add partial expert outputs ----
    out = x_flat.copy()
    for e in range(E):
        yg = res.results[e]["ygt"]                          # [C, NT]
        n = len(idxs[e])
        out[idxs[e]] += yg.T[:n]
    return out.reshape(B, T, C).astype(np.float32)


# revision 23
# speedup vs baseline: 1.1194x; 1.0636x over previous
"""Trainium2 Bass kernel for CaMoE (LN + top-2 MoE with relu^2 FFN).

Strategy: expert-parallel over 8 NeuronCores. Core e receives only the
tokens routed to expert e (gathered host-side), sorted by DESCENDING
combine coefficient, plus W1[e]/W2[e] pre-swizzled into per-tile lhsT
layout. Low-coefficient token blocks run their matmuls in fp8-e4m3 with
perf_mode=DoubleRow (2 K-subtiles per instruction, ~1.8x the bf16 PE
rate); high-coefficient blocks stay bf16. The routing coefficient folds
into the LN scale as sqrt(coef) (relu^2 is 2-homogeneous), so a block's
fp8 quantization error is damped by its (small) coef - that keeps the
absmax error within budget while ~half the FLOPs run at fp8 rate.

LayerNorm stats come from ones-matmuls of fp8 copies of x (DoubleRow as
well); their contribution to the error is negligible. On-chip engines
are balanced: ReLU on ScalarE (with the fp8 scale folded), the square +
down-cast and the normalize on VectorE in 2x bf16 mode, matmuls on PE.

Host scatter-adds the 8 partial outputs into x (the residual) - pure
unsharding, no collectives.

Self-contained: hardcodes shapes B=4, T=2048, C=1024, E=8, H=4096.
"""

import os
import sys

for _p in ("/opt/trn_rl_repo", "/root/.axon_site/_ro/trn_rl_repo"):
    if os.path.isdir(_p) and _p not in sys.path:
        sys.path.insert(0, _p)

from contextlib import ExitStack

import ml_dtypes
import numpy as np

import concourse.bass as bass
import concourse.tile as tile
from concourse import bacc, mybir
from concourse.bass_utils import run_bass_kernel_spmd

N_CORES = 8
C = 1024
H = 4096
NB = 512          # token block (matmul moving free dim)
NC_T = C // 128   # 8 c-tiles
NH_T = H // 128   # 32 h-tiles
EPS = 1e-5
SW = 64.0         # fp8 weight scale (both W1 and W2)
# fp8 tier: rt = sqrt(2)/SW * relu(pa) so ht = rt^2 = 2*h; out = pb/(2*SW)

F32 = mybir.dt.float32
BF16 = mybir.dt.bfloat16
FP8 = mybir.dt.float8e4
AF = mybir.ActivationFunctionType
OP = mybir.AluOpType
DR = mybir.MatmulPerfMode.DoubleRow

# number of trailing (lowest-coef) 512-token blocks run fully in fp8, and
# number of "mm2"-tier blocks (bf16 mm1 + fp8 mm2) just before those
N_FP8_BLOCKS = 2
N_MM2_BLOCKS = 0


def _build_kernel(NT: int, tiers: tuple, has_beta: bool):
    """Build the per-core SPMD program for NT padded tokens.

    tiers[b] in {"bf16", "full"}: precision of block b's matmuls.
    """
    blocks = []
    t0 = 0
    while t0 < NT:
        tn = min(NB, NT - t0)
        blocks.append((t0, tn))
        t0 += tn
    nblk = len(blocks)
    assert len(tiers) == nblk
    any_f8_1 = any(t == "full" for t in tiers)
    any_f8_2 = any(t in ("full", "mm2") for t in tiers)
    any_bf_1 = any(t in ("bf16", "mm2") for t in tiers)
    any_bf_2 = any(t == "bf16" for t in tiers)

    nc = bacc.Bacc("TRN2", target_bir_lowering=False, debug=False, num_devices=1)

    # x stored feature-major; declared pair-of-c-tile shaped so one DMA
    # fills a [128, 2, tn] SBUF tile
    xgt_d = nc.dram_tensor("xgt", [NC_T // 2, 2, 128, NT], F32,
                           kind="ExternalInput").ap()
    # weights pre-swizzled on host into per-tile lhsT layout:
    #   w1[h][p, c, j] = (gamma*W1)[c*128+p, h*128+j]   (fp8 copy scaled by SW)
    #   w2[c][p, h, j] = W2[h*128+p, c*128+j]
    if any_bf_1:
        w1b_d = nc.dram_tensor("w1b", [NH_T, 128, NC_T, 128], BF16,
                               kind="ExternalInput").ap()
    if any_bf_2:
        w2b_d = nc.dram_tensor("w2b", [NC_T, 128, NH_T, 128], BF16,
                               kind="ExternalInput").ap()
    if any_f8_1:
        w1f_d = nc.dram_tensor("w1f", [NH_T, 128, NC_T, 128], FP8,
                               kind="ExternalInput").ap()
    if any_f8_2:
        w2f_d = nc.dram_tensor("w2f", [NC_T, 128, NH_T, 128], FP8,
                               kind="ExternalInput").ap()
    cg_d = nc.dram_tensor("cg", [1, NT], F32, kind="ExternalInput").ap()
    if has_beta:
        bias1_d = nc.dram_tensor("bias1", [128, NH_T], F32, kind="ExternalInput").ap()
    ygt_d = nc.dram_tensor("ygt", [C, NT], F32, kind="ExternalOutput").ap()

    with tile.TileContext(nc) as tc, ExitStack() as ctx:
        sb = ctx.enter_context(tc.tile_pool(name="sb", bufs=1))
        ps = ctx.enter_context(tc.tile_pool(name="ps", bufs=1, space="PSUM"))

        # ---- constants ----
        ones8 = sb.tile([128, 2, 128], FP8, tag="ones8", bufs=1)
        nc.vector.memset(ones8, 1.0)
        eps_t = sb.tile([128, 1], F32, tag="eps", bufs=1)
        nc.vector.memset(eps_t, EPS)
        if has_beta:
            b1sb = sb.tile([128, NH_T], F32, tag="b1", bufs=1)
            nc.sync.dma_start(b1sb, bias1_d)

        def stats_dma(blk):
            """Kick the x DMAs for block blk (one per c-tile pair)."""
            t0, tn = blocks[blk]
            tsl = bass.ds(t0, tn)
            xs = []
            for i in range(NC_T // 2):
                xt = sb.tile([128, 2, tn], F32, tag="xs", bufs=8, name=f"xa{blk}_{i}",
                             padded_shape=[128, 2, NB])
                nc.sync.dma_start(xt, xgt_d[i][:, :, tsl].transpose([1, 0, 2]))
                xs.append(xt)
            return xs

        def stats_phase(blk, xs):
            """LN stats for block blk via fp8 DoubleRow ones-matmuls.

            Returns f32 [128,tn] scale/shift (broadcast across
            partitions)."""
            t0, tn = blocks[blk]
            tsl = bass.ds(t0, tn)
            sum_ps = ps.tile([128, tn], F32, tag="stat", bufs=3, name=f"sum{blk}")
            sq_ps = ps.tile([128, tn], F32, tag="stat", bufs=3, name=f"sq{blk}")
            xb = sb.tile([128, NC_T, tn], FP8, tag="xb", bufs=2, name=f"xb{blk}",
                         padded_shape=[128, NC_T, NB])
            xq = sb.tile([128, NC_T, tn], FP8, tag="xq", bufs=2, name=f"xq{blk}",
                         padded_shape=[128, NC_T, NB])
            for i in range(NC_T // 2):
                pr = bass.ds(2 * i, 2)
                nc.vector.tensor_copy(xb[:, pr, :], xs[i])
                nc.scalar.activation(xq[:, pr, :], xs[i], AF.Square)
            for i in range(NC_T // 2):
                pr = bass.ds(2 * i, 2)
                nc.tensor.matmul(sum_ps, ones8, xb[:, pr, :], perf_mode=DR,
                                 start=(i == 0), stop=(i == NC_T // 2 - 1))
            for i in range(NC_T // 2):
                pr = bass.ds(2 * i, 2)
                nc.tensor.matmul(sq_ps, ones8, xq[:, pr, :], perf_mode=DR,
                                 start=(i == 0), stop=(i == NC_T // 2 - 1))
            vmu = sb.tile([128, tn], F32, tag="vec", bufs=3, name=f"vmu{blk}",
                          padded_shape=[128, NB])
            nc.vector.tensor_scalar_mul(vmu, sum_ps, 1.0 / C)
            # var = sq/C - mu^2
            vvar = sb.tile([128, tn], F32, tag="vec", bufs=3, name=f"vvar{blk}",
                           padded_shape=[128, NB])
            nc.vector.scalar_tensor_tensor(vvar, vmu, -1.0, vmu, OP.mult, OP.mult)
            nc.vector.scalar_tensor_tensor(vvar, sq_ps, 1.0 / C, vvar, OP.mult, OP.add)
            vstd = sb.tile([128, tn], F32, tag="vec", bufs=3, name=f"vstd{blk}",
                           padded_shape=[128, NB])
            nc.scalar.activation(vstd, vvar, AF.Sqrt, bias=eps_t)
            vrstd = sb.tile([128, tn], F32, tag="vec", bufs=3, name=f"vrstd{blk}",
                            padded_shape=[128, NB])
            nc.vector.reciprocal_approx_fast(out=vrstd, in_=vstd)
            vcg = sb.tile([128, tn], F32, tag="bc", bufs=4, name=f"vcg{blk}",
                          padded_shape=[128, NB])
            nc.sync.dma_start(vcg, cg_d[0:1, tsl].to_broadcast([128, tn]))
            if has_beta:
                vs = vrstd                         # coef applied on the output
            else:
                vs = sb.tile([128, tn], F32, tag="bc", bufs=4, name=f"vs{blk}",
                             padded_shape=[128, NB])
                nc.vector.tensor_mul(vs, vrstd, vcg)
            vb = sb.tile([128, tn], F32, tag="bc", bufs=4, name=f"vb{blk}",
                         padded_shape=[128, NB])
            nc.vector.scalar_tensor_tensor(vb, vmu, -1.0, vs, OP.mult, OP.mult)
            return vs, vb, vcg

        def normalize_phase(blk, vs, vb, xs):
            t0, tn = blocks[blk]
            f8 = tiers[blk] == "full"            # mm1 precision
            xn = sb.tile([128, NC_T, tn], FP8 if f8 else BF16,
                         tag="xn8" if f8 else "xnb", bufs=2, name=f"xn{blk}",
                         padded_shape=[128, NC_T, NB])
            for c in range(NC_T):
                xt = xs[c // 2][:, c % 2, :]
                nc.vector.tensor_mul(xt, xt, vs)
                nc.vector.tensor_add(xn[:, c, :], xt, vb)
            return xn

        def mm1_phase(blk, xn, hooks=()):
            t0, tn = blocks[blk]
            f8_1 = tiers[blk] == "full"
            f8_2 = tiers[blk] in ("full", "mm2")
            hid = sb.tile([128, NH_T, tn], FP8 if f8_2 else BF16,
                          tag="hid8" if f8_2 else "hidb", bufs=1, name=f"hid{blk}",
                          padded_shape=[128, NH_T, NB])
            # fp8 tiers: rt = sqrt(2)*relu(a) so hid = rt^2 = 2h (in fp8
            # range); mm1-fp8 additionally divides out the W1 scale SW.
            rs = 1.0 if not f8_2 else (np.sqrt(2.0) / SW if f8_1 else np.sqrt(2.0))
            for hp in range(NH_T // 2):
                for at, hook in hooks:
                    if hp == at:
                        hook()
                # two h-tiles share one 2-bank PSUM tile so the DVE ops
                # below run once per pair at [128, 2*tn]
                pa = ps.tile([128, 2, tn], F32, tag="mm", bufs=2,
                             name=f"pa{blk}_{hp}", padded_shape=[128, 2, NB])
                for j in range(2):
                    h = 2 * hp + j
                    if f8_1:
                        w1t = sb.tile([128, NC_T, 128], FP8, tag="w1f", bufs=4,
                                      name=f"w1f{blk}_{h}")
                        nc.sync.dma_start(w1t, w1f_d[h])
                        for i in range(NC_T // 2):
                            nc.tensor.matmul(pa[:, j, :],
                                             w1t[:, bass.ds(2 * i, 2), :],
                                             xn[:, bass.ds(2 * i, 2), :],
                                             perf_mode=DR, start=(i == 0),
                                             stop=(i == NC_T // 2 - 1))
                    else:
                        w1t = sb.tile([128, NC_T, 128], BF16, tag="w1b", bufs=4,
                                      name=f"w1b{blk}_{h}")
                        nc.sync.dma_start(w1t, w1b_d[h])
                        for c in range(NC_T):
                            nc.tensor.matmul(pa[:, j, :], w1t[:, c, :], xn[:, c, :],
                                             start=(c == 0), stop=(c == NC_T - 1))
                if has_beta:
                    for j in range(2):
                        nc.vector.tensor_scalar_add(pa[:, j, :], pa[:, j, :],
                                                    b1sb[:, 2 * hp + j:2 * hp + j + 1])
                rt = sb.tile([128, 2, tn], BF16, tag="rt", bufs=3, name=f"r{blk}_{hp}",
                             padded_shape=[128, 2, NB])
                nc.vector.tensor_scalar(rt, pa, 0.0, rs, OP.max, OP.mult)
                nc.vector.tensor_mul(hid[:, bass.ds(2 * hp, 2), :], rt, rt)
            return hid

        def mm2_phase(blk, hid, vcf):
            t0, tn = blocks[blk]
            tsl = bass.ds(t0, tn)
            f8 = tiers[blk] in ("full", "mm2")
            oscale = 1.0 / (2.0 * SW) if f8 else 1.0
            for cp in range(NC_T // 2):
                pb = ps.tile([128, 2, tn], F32, tag="mm", bufs=2,
                             name=f"pb{blk}_{cp}", padded_shape=[128, 2, NB])
                for j in range(2):
                    c = 2 * cp + j
                    if f8:
                        w2t = sb.tile([128, NH_T, 128], FP8, tag="w2f", bufs=2,
                                      name=f"w2f{blk}_{c}")
                        nc.sync.dma_start(w2t, w2f_d[c])
                        for i in range(NH_T // 2):
                            nc.tensor.matmul(pb[:, j, :],
                                             w2t[:, bass.ds(2 * i, 2), :],
                                             hid[:, bass.ds(2 * i, 2), :],
                                             perf_mode=DR, start=(i == 0),
                                             stop=(i == NH_T // 2 - 1))
                    else:
                        w2t = sb.tile([128, NH_T, 128], BF16, tag="w2b", bufs=2,
                                      name=f"w2b{blk}_{c}")
                        nc.sync.dma_start(w2t, w2b_d[c])
                        for i in range(NH_T):
                            nc.tensor.matmul(pb[:, j, :], w2t[:, i, :], hid[:, i, :],
                                             start=(i == 0), stop=(i == NH_T - 1))
                ot = sb.tile([128, 2, tn], F32, tag="out", bufs=2, name=f"o{blk}_{cp}",
                             padded_shape=[128, 2, NB])
                if has_beta:
                    for j in range(2):
                        nc.vector.scalar_tensor_tensor(ot[:, j, :], pb[:, j, :],
                                                       oscale, vcf, OP.mult, OP.mult)
                else:
                    nc.vector.tensor_scalar_mul(ot, pb, oscale)
                for j in range(2):
                    c = 2 * cp + j
                    nc.sync.dma_start(ygt_d[c * 128:(c + 1) * 128, tsl], ot[:, j, :])

        # Software pipeline: x DMAs of blk+1 kick off early in blk's mm1;
        # stats of blk+1 are emitted mid-mm1 so the PE runs them inside
        # blk's matmul stream; normalize of blk+1 lands before blk's mm2.
        xs0 = stats_dma(0)
        vs0, vb0, vcf = stats_phase(0, xs0)
        xn = normalize_phase(0, vs0, vb0, xs0)
        nxt = {}
        for blk in range(nblk):
            hooks = []
            if blk + 1 < nblk:
                def dma_hook(b=blk):
                    nxt["xs"] = stats_dma(b + 1)

                def stat_hook(b=blk):
                    nxt.update(zip(("vs", "vb", "vcf"), stats_phase(b + 1, nxt["xs"])))
                hooks = [(1, dma_hook), (8, stat_hook)]
            hid = mm1_phase(blk, xn, hooks)
            if blk + 1 < nblk:
                xn = normalize_phase(blk + 1, nxt["vs"], nxt["vb"], nxt["xs"])
            mm2_phase(blk, hid, vcf)
            if blk + 1 < nblk:
                vcf = nxt["vcf"]

    nc.compile()
    return nc


_KERNEL_CACHE = {}


def _get_kernel(NT: int, tiers: tuple, has_beta: bool):
    key = (NT, tiers, has_beta)
    if key not in _KERNEL_CACHE:
        _KERNEL_CACHE[key] = _build_kernel(NT, tiers, has_beta)
    return _KERNEL_CACHE[key]


def _swizzle_w1(w, dtype):
    # [C, H] -> [NH_T, 128, NC_T, 128] with [h][p, c, j] = w[c*128+p, h*128+j]
    return np.ascontiguousarray(
        w.reshape(NC_T, 128, NH_T, 128).transpose(2, 1, 0, 3)
    ).astype(dtype)


def _swizzle_w2(w, dtype):
    # [H, C] -> [NC_T, 128, NH_T, 128] with [c][p, h, j] = w[h*128+p, c*128+j]
    return np.ascontiguousarray(
        w.reshape(NH_T, 128, NC_T, 128).transpose(2, 1, 0, 3)
    ).astype(dtype)


def kernel(x, weights, gamma, beta, W1, W2, winners):
    x = np.asarray(x, dtype=np.float32)
    weights = np.asarray(weights, dtype=np.float32)
    gamma = np.asarray(gamma, dtype=np.float32)
    beta = np.asarray(beta, dtype=np.float32)
    W1 = np.asarray(W1, dtype=np.float32)
    W2 = np.asarray(W2, dtype=np.float32)
    winners = np.asarray(winners)

    B, T, C_ = x.shape
    E = W1.shape[0]
    assert C_ == C and E == N_CORES and W1.shape[2] == H

    x_flat = x.reshape(-1, C)
    win = winners.reshape(-1, 2)
    wts = weights.reshape(-1, 2)

    has_beta = bool(np.any(beta != 0.0))

    # ---- host-side routing (sharding prep) ----
    idxs, coefs = [], []
    for e in range(E):
        m = win == e
        tok = np.nonzero(m.any(axis=1))[0]
        cf = (wts * m).sum(axis=1)[tok]
        order = np.argsort(-cf, kind="stable")   # descending coef
        idxs.append(tok[order])
        coefs.append(cf[order].astype(np.float32))
    NT = int(np.ceil(max(len(t) for t in idxs) / 8) * 8)
    nblk = (NT + NB - 1) // NB

    # trailing (low-coef) blocks in fp8, unless beta forces plain path
    n_f8 = 0 if has_beta else min(N_FP8_BLOCKS, nblk)
    n_m2 = 0 if has_beta else min(N_MM2_BLOCKS, nblk - n_f8)
    tiers = tuple(["bf16"] * (nblk - n_f8 - n_m2) + ["mm2"] * n_m2
                  + ["full"] * n_f8)
    any_f8_1 = "full" in tiers
    any_f8_2 = n_f8 + n_m2 > 0
    any_bf_1 = nblk - n_f8 > 0
    any_bf_2 = nblk - n_f8 - n_m2 > 0

    in_maps = []
    for e in range(E):
        tok, cf = idxs[e], coefs[e]
        n = len(tok)
        xg = np.zeros((NT, C), np.float32)
        xg[:n] = x_flat[tok]
        cg = np.zeros((1, NT), np.float32)
        # no beta: fold sqrt(coef) into the LN scale (relu^2 is 2-homogeneous
        # and W2 linear, so scaling xn by sqrt(c) scales the output by c).
        cg[0, :n] = cf if has_beta else np.sqrt(cf)
        w1g = W1[e] * gamma[:, None]
        m = {
            "xgt": np.ascontiguousarray(xg.T).reshape(NC_T // 2, 2, 128, NT),
            "cg": cg,
        }
        if any_bf_1:
            m["w1b"] = _swizzle_w1(w1g, ml_dtypes.bfloat16)
        if any_bf_2:
            m["w2b"] = _swizzle_w2(W2[e], ml_dtypes.bfloat16)
        if any_f8_1:
            m["w1f"] = _swizzle_w1(w1g * SW, ml_dtypes.float8_e4m3)
        if any_f8_2:
            m["w2f"] = _swizzle_w2(W2[e] * SW, ml_dtypes.float8_e4m3)
        if has_beta:
            b1 = (beta @ W1[e]).astype(np.float32)          # [H]
            m["bias1"] = np.ascontiguousarray(b1.reshape(NH_T, 128).T)
        in_maps.append(m)

    nc = _get_kernel(NT, tiers, has_beta)
    res = run_bass_kernel_spmd(nc, in_maps, list(range(N_CORES)))

    # ---- host-side unshard: scatter-add partial expert outputs ----
    out = x_flat.copy()
    for e in range(E):
        yg = res.results[e]["ygt"]                          # [C, NT]
        n = len(idxs[e])
        out[idxs[e]] += yg.T[:n]
    return out.reshape(B, T, C).astype(np.float32)
